# revision 6
# baseline (speedup 1.0000x reference)
"""Trainium2 Bass kernel for nn_Attention (channel-attention, 8 NeuronCores).

Algorithm (algebraically identical to the reference):
  The attention contracts over the spatial axis n = 32*32*32 = 32768, and the
  attention matrices are tiny (64x64 per head).  Everything collapses around
  the per-batch Gram matrix G_b = x_b @ x_b^T (128x128):

    scores_bh = scale * Wq_h G_b Wk_h^T            (tiny)
    attn      = softmax(scores)                     (tiny)
    W_eff_b   = (1/n) * sum_h Wout_h attn_bh Wv_h   (64x128, tiny)
    y_b       = W_eff_b @ x_b + b_out               (the only other big matmul)

  Sharding: NO collectives (an ncfw collective costs 60-80us of firmware
  wakeup on this stack, dwarfing the kernel).  Every core receives the FULL
  x in fp8-e4m3 [n, c] layout (8 MB) and computes the complete Gram
  redundantly (fp8 is harmless: the Gram contracts over 32768 samples), plus
  its own 1/8 spatial shard in bf16 [c, n] layout (2 MB) for the y matmul.

  Performance model (from perfetto/NTFF analysis of the previous version):
  - The input stream is the wall: ~10.9 MB at ~360 GB/s = ~30.5 us.
  - The PE at full clock consumes a fp8-DR Gram pair (256 spatial rows) every
    ~78 ns => 20 us of Gram work, comfortably inside the stream...
  - ...BUT the HW power manager demotes the PE to half clock (HAM k=4/8
    windows in the NTFF) after idle gaps, and re-promotes slowly.  The old
    version starved the PE at chunk boundaries and during softmax waits,
    lost the full clock for ~30 us of the run, and finished the Gram ~13 us
    after the stream ended.
  Fixes here:
  - 256KB-piece-granular streaming (2 pieces per 512 KB chunk) so the PE
    never waits more than ~0.3 us for data.  Pieces stay >=256 KB because
    descriptor issue costs ~0.6 us each on the sync/scalar queues: smaller
    pieces make the stream issue-bound (measured).
  - Zero-data warm matmuls (fp8 DR on a memset tile: no switching power)
    fill the three unavoidable PE gaps: DMA prefill, the b0->b1 stream
    boundary (softmax0 wait), and the softmax1 wait on the tail.
  - The softmax is pipelined per head-group: ACT exp carries bias=-max and
    accum_out=row-sum in ONE instruction, so the chain per group is
    DVE(max) -> ACT(exp+sum) -> DVE(recip, scale) -> PE(mt), overlapped
    across the 4 groups.
  - Phase E packs two 512-col output chunks into one [128, 512] PSUM tile
    (PE quadrant packing), halving the bias-add and output-DMA count.
  - Queue routing: sync+scalar carry only the input stream (gram pieces,
    then xc last so the tail-needed shard arrives exactly at stream end)
    plus the batch-1 outputs (post-stream, FIFO-safe); gpsimd SWDGE carries
    wpack and batch-0 outputs so they never head-of-line block the stream.
"""

import numpy as np
import ml_dtypes

import concourse.bass as bass
import concourse.bacc as bacc
import concourse.mybir as mybir
import concourse.tile as tile
from concourse.bass_utils import run_bass_kernel_spmd

NCORES = 8
P = 128
N_TOT = 32 * 32 * 32          # 32768 spatial points
NSH = N_TOT // NCORES         # 4096 per core per batch (output shard)
SUB = N_TOT // P              # 256 fp8 k-subtiles per batch
CHUNK_SUB = 32                # subtiles per DMA chunk (512 KB)
NCHUNK = SUB // CHUNK_SUB     # 8 chunks per batch
CHW = CHUNK_SUB * P           # 4096 fp8 free columns per chunk
PIECES = 2                    # DMA pieces per chunk (256 KB each)
HEADS = 8
DH = 64
SCALE = DH ** -0.5
WCOLS = 512 + 512 + 512 + 256 + 1  # packed weights: wq|wk|wv|wo|bo
WARM_START = 0                # PE warm-keepers (OFF: the HW throttle is a
WARM_MID = 0                  # utilization budget -- idle EARNS credit, so
WARM_TAIL = 0                 # fillers burn it and stretch the run)
BF = mybir.dt.bfloat16
F32 = mybir.dt.float32
FP8 = mybir.dt.float8e4
DR = mybir.MatmulPerfMode.DoubleRow
DRSW = mybir.MatmulPerfMode.DoubleRowSwInterleave
EXP = mybir.ActivationFunctionType.Exp
bf16 = ml_dtypes.bfloat16
f8 = ml_dtypes.float8_e4m3

_CACHED_NC = None


class _TrimmedTileContext(tile.TileContext):
    """TileContext with a minimal exit sequence.

    The stock exit is drain -> barrier -> sem-clear -> barrier; the
    barrier + clear lower to an EVSEM butterfly measured at ~7us (every
    engine walks the 27-sem global clock).  For a single-shot kernel the
    Sync drain with global-clock waits already gates completion on every
    DMA and engine op, each engine halts in-order after its last
    scheduled instruction, and the engine preamble re-initializes the
    semaphore file on the next execution (verified: back-to-back
    executions of the same loaded NEFF stay correct).  So keep only the
    drain.
    """

    def _drain_and_barrier(self, tick_clock, wait_clock):
        from concourse.vector_clock import ScopedClock

        drain_inst = self.nc.sync.drain()
        wait_clock.add_sem_waits(
            drain_inst.ins, ScopedClock({None: tick_clock.global_clock})
        )
        popped = self.nc._tile_sem_poison_stack.pop()
        assert popped is self._sem_poison


def build_nc():
    # The stock Bass init ends with const-AP memsets guarded by a second
    # all-engine barrier; the consts are unused here and the barrier adds
    # ~2us of start-up serialization, so skip that one barrier only.
    orig_barrier = bass.Bass.all_engine_barrier
    bass.Bass.all_engine_barrier = lambda self: None
    try:
        nc = bacc.Bacc(
            "TRN2", target_bir_lowering=False, debug=False, num_devices=NCORES
        )
    finally:
        bass.Bass.all_engine_barrier = orig_barrier

    # full x, fp8, [p, (b, m, c)] DoubleRow layout: subtile m holds spatial
    # rows m*128..m*128+127 of batch b, channels on the innermost axis.
    xg_ext = nc.dram_tensor("xg", [P, 2 * SUB * P], FP8, kind="ExternalInput")
    # own output shard, bf16, [c, (b, n)] layout for the y matmul
    xc_ext = nc.dram_tensor("xc", [P, 2 * NSH], BF, kind="ExternalInput")
    w_ext = nc.dram_tensor("wpack", [P, WCOLS], BF, kind="ExternalInput")
    # y out, bf16: partition = (chunk-half, row), free = (b, pair, 512)
    out_ext = nc.dram_tensor("out", [P, NSH], BF, kind="ExternalOutput")

    with _TrimmedTileContext(nc) as tc:
        with (
            tc.tile_pool(name="const", bufs=1) as const,
            tc.tile_pool(name="data", bufs=1) as data,
            tc.tile_pool(name="work", bufs=1) as work,
            tc.tile_pool(name="ypool", bufs=8) as ypool,
            tc.tile_pool(name="psg", bufs=2, space="PSUM") as psg,
            tc.tile_pool(name="psd", bufs=2, space="PSUM") as psd,
            tc.tile_pool(name="psy", bufs=3, space="PSUM") as psy,
            tc.tile_pool(name="psw", bufs=1, space="PSUM") as psw,
        ):
            # ---- input DMAs: program order == ring FIFO order ----
            # sync+scalar rings carry ONLY the stream, piece-interleaved:
            # b0 gram, xc0, b1 gram, xc1.  Each 512KB chunk is split into
            # two 256KB pieces on opposite rings so both rings work on the
            # same chunk and the PE's per-piece waits stay ~0.35us.
            xg_tiles = [[], []]
            qs = [nc.sync, nc.scalar]

            def make_xg(b, c):
                t = data.tile([P, CHW], FP8, tag=f"xg{b}_{c}")
                off = (b * SUB + c * CHUNK_SUB) * P
                pw = CHW // PIECES
                for p in range(PIECES):
                    qs[p % 2].dma_start(
                        t[:, p * pw : (p + 1) * pw],
                        xg_ext[:, off + p * pw : off + (p + 1) * pw],
                    )
                xg_tiles[b].append(t)

            xc = data.tile([P, 2 * NSH], BF, tag="xc")
            wpack = const.tile([P, WCOLS], BF, tag="wpack")
            wq = wpack[:, 0:512]
            wk = wpack[:, 512:1024]
            wv = wpack[:, 1024:1536]
            wo = wpack[:, 1536:1792]

            # wpack rides the gpsimd SWDGE ring: needed mid-stream, and it
            # must not displace gram bytes at the head of the hw rings.
            nc.gpsimd.dma_start(wpack[:], w_ext[:])

            for c in range(NCHUNK):
                make_xg(0, c)
            # xc(b0) after b0's gram: arrives before E0 needs it, after the
            # whole b0 Gram is on chip.
            nc.sync.dma_start(xc[:, 0 : NSH // 2], xc_ext[:, 0 : NSH // 2])
            nc.scalar.dma_start(xc[:, NSH // 2 : NSH], xc_ext[:, NSH // 2 : NSH])
            for c in range(NCHUNK):
                make_xg(1, c)
            # xc(b1) LAST: the tail (phase E1) is the only consumer, and the
            # D1 attention chain fully covers its transfer.
            nc.sync.dma_start(
                xc[:, NSH : NSH + NSH // 2], xc_ext[:, NSH : NSH + NSH // 2]
            )
            nc.scalar.dma_start(
                xc[:, NSH + NSH // 2 :], xc_ext[:, NSH + NSH // 2 :]
            )

            # ---- constants / warm fodder ----
            bo = work.tile([P, 1], F32, tag="bo")
            dummy = work.tile([P, 2 * P], FP8, tag="dummy")
            nc.vector.memset(dummy[:], 0.0)
            nc.vector.tensor_copy(bo[:], wpack[:, 1792:1793])
            dummy_r = dummy[:].rearrange("p (m c) -> p m c", c=P)

            # Zero-data fp8 DR matmuls: keep the PE's activity up (the HW
            # power manager demotes the clock after idle gaps) without
            # meaningful switching power.  ~78ns each at full clock.
            warm_ps = psw.tile([P, P], F32, tag="warm")

            def warm(n):
                for _ in range(n):
                    nc.tensor.matmul(
                        warm_ps[:], dummy_r, dummy_r,
                        start=True, stop=True, perf_mode=DR,
                    )

            # ---- Gram accumulation (fp8 DoubleRow) ----
            g_ps = [None, None]
            gbf = [None, None]

            def gram_chunks(b, c_lo, c_hi):
                if g_ps[b] is None:
                    g_ps[b] = psg.tile([P, P], F32, tag="g", name=f"g_ps{b}")
                n_mm = CHUNK_SUB // 2
                for c in range(c_lo, c_hi):
                    xr = xg_tiles[b][c][:].rearrange("p (m q) -> p m q", q=2 * P)
                    for j in range(n_mm):
                        # software-interleaved pair block: per partition the
                        # 256 bytes are [A_c127, B_c127, ..., A_c0, B_c0]
                        # (A/B = the two k-subtiles, columns reversed per the
                        # HW SwInterleave contract).  The weights AP streams
                        # the storage order; the ifmap AP picks plane i at
                        # stride 2.  G comes out with reversed columns,
                        # absorbed by reversing wk's rows host-side.
                        blk = xr[:, j, :]
                        lhsT = blk.rearrange("p (qq two) -> p qq two", two=2)
                        rhs = blk.rearrange("p (qq two) -> p two qq", two=2)
                        nc.tensor.matmul(
                            g_ps[b][:], lhsT, rhs,
                            start=(c == 0 and j == 0),
                            stop=(c == NCHUNK - 1 and j == n_mm - 1),
                            perf_mode=DRSW,
                        )

            # ---- phase D: scores (PE), softmax (DVE/ACT), W_eff (PE) ----
            s_tiles = {}

            def d_scores(b):
                """gbf cast; a = G Wq; S_h = a_h^T Wk_h (quadrant-packed)."""
                gbf[b] = work.tile([P, P], BF, tag=f"gbf{b}", name=f"gbf{b}")
                nc.vector.tensor_copy(gbf[b][:], g_ps[b][:])
                a_ps = psd.tile([P, 512], F32, tag="d", name=f"a_ps{b}")
                a_sb = work.tile([P, 512], BF, tag=f"asb{b}", name=f"a_sb{b}")
                s_ps = psd.tile([P, 256], F32, tag="d", name=f"s_ps{b}")
                nc.tensor.matmul(a_ps[:], gbf[b][:], wq, start=True, stop=True)
                for sl in range(4):
                    nc.vector.tensor_copy(
                        a_sb[:, sl * 128 : (sl + 1) * 128],
                        a_ps[:, sl * 128 : (sl + 1) * 128],
                    )
                for h in range(HEADS):
                    pb = 64 * (h % 2)
                    cg = 64 * (h // 2)
                    nc.tensor.matmul(
                        s_ps[pb : pb + 64, cg : cg + 64],
                        a_sb[:, h * 64 : (h + 1) * 64],
                        wk[:, h * 64 : (h + 1) * 64],
                        start=True, stop=True,
                    )
                s_tiles[b] = s_ps

            def d_softmax(b):
                """Per-group: exp(s - max) with fused row-sum, then scale.

                ACT Exp takes bias = -max (per-partition AP) and emits the
                row sum via accum_out in the same instruction, so the chain
                is DVE(max) -> ACT(exp+sum) -> DVE(recip) -> DVE(scale),
                pipelined across the 4 head-groups.
                """
                s_ps = s_tiles[b]
                negmax = work.tile([P, 4], F32, tag=f"nm{b}", name=f"negmax{b}")
                exp_sb = work.tile([P, 256], F32, tag=f"exp{b}", name=f"exp_sb{b}")
                sums = work.tile([P, 4], F32, tag=f"sums{b}", name=f"sums{b}")
                recip = work.tile([P, 4], F32, tag=f"recip{b}", name=f"recip{b}")
                attn = work.tile([P, 256], BF, tag=f"attn{b}", name=f"attn{b}")
                nc.vector.reduce_max(
                    negmax[:],
                    s_ps[:].rearrange("p (g j) -> p g j", j=64),
                    axis=mybir.AxisListType.X,
                    negate=True,
                )
                for g in range(4):
                    cg = 64 * g
                    nc.scalar.activation(
                        exp_sb[:, cg : cg + 64],
                        s_ps[:, cg : cg + 64],
                        EXP,
                        bias=negmax[:, g : g + 1],
                        scale=1.0,
                        accum_out=sums[:, g : g + 1],
                    )
                    nc.vector.reciprocal(recip[:, g : g + 1], sums[:, g : g + 1])
                    nc.vector.tensor_scalar_mul(
                        attn[:, cg : cg + 64],
                        exp_sb[:, cg : cg + 64],
                        recip[:, g : g + 1],
                    )
                return attn

            def d_weff(b, attn):
                """MT_h = attn_h^T WoT_h; W_eff = wv MT (group-pipelined)."""
                mt_ps = psd.tile([P, 256], F32, tag="d", name=f"mt_ps{b}")
                mt_sb = work.tile([P, 256], BF, tag=f"mt{b}", name=f"mt_sb{b}")
                w_ps = psd.tile([P, 64], F32, tag="d", name=f"w_ps{b}")
                weff = work.tile([P, 64], BF, tag=f"weff{b}", name=f"weff{b}")
                for g in range(4):
                    cg = 64 * g
                    for pb in (0, 64):
                        nc.tensor.matmul(
                            mt_ps[pb : pb + 64, cg : cg + 64],
                            attn[pb : pb + 64, cg : cg + 64],
                            wo[pb : pb + 64, cg : cg + 64],
                            start=True, stop=True,
                        )
                    nc.vector.tensor_copy(
                        mt_sb[:, cg : cg + 64], mt_ps[:, cg : cg + 64]
                    )
                    nc.tensor.matmul(
                        w_ps[:],
                        wv[:, g * P : (g + 1) * P],
                        mt_sb[:, cg : cg + 64],
                        start=(g == 0), stop=(g == 3),
                    )
                nc.vector.tensor_copy(weff[:], w_ps[:])
                return weff

            def phase_e(b, weff, t_lo, t_hi):
                """y_b = W_eff_b @ x_b + b_out, two 512-col chunks per PSUM
                tile via quadrant packing (out partitions 0-63 / 64-127)."""
                for t in range(t_lo, t_hi):
                    y_ps = psy.tile([P, 512], F32, tag="y", name=f"y_ps{b}_{t}")
                    for half in (0, 1):
                        j = 2 * t + half
                        nc.tensor.matmul(
                            y_ps[64 * half : 64 * half + 64, :],
                            weff[:],
                            xc[:, b * NSH + j * 512 : b * NSH + (j + 1) * 512],
                            start=True, stop=True,
                        )
                    y_sb = ypool.tile([P, 512], BF, tag="ysb", name=f"y_sb{b}_{t}")
                    nc.vector.tensor_scalar_add(y_sb[:], y_ps[:], bo[:, 0:1])
                    dst = out_ext[:, (b * 4 + t) * 512 : (b * 4 + t + 1) * 512]
                    if b == 0:
                        # mid-stream: SWDGE so the hw rings keep streaming
                        nc.gpsimd.dma_start(dst, y_sb[:])
                    else:
                        # post-stream: hw rings are drained, FIFO-safe
                        qs[t % 2].dma_start(dst, y_sb[:])

            # ---- PE program order ----
            warm(WARM_START)
            gram_chunks(0, 0, NCHUNK)
            d_scores(0)
            attn0 = d_softmax(0)     # DVE/ACT; PE covered by warms below
            warm(WARM_MID)           # b0->b1 stream boundary (xc0 transfer)
            gram_chunks(1, 0, 2)
            weff0 = d_weff(0, attn0)
            gram_chunks(1, 2, 5)
            phase_e(0, weff0, 0, 2)
            gram_chunks(1, 5, 6)
            phase_e(0, weff0, 2, 4)
            gram_chunks(1, 6, NCHUNK)
            d_scores(1)
            attn1 = d_softmax(1)
            warm(WARM_TAIL)          # softmax1 wait on the tail
            weff1 = d_weff(1, attn1)
            phase_e(1, weff1, 0, 4)

    nc.compile()
    return nc


def _get_nc():
    global _CACHED_NC
    if _CACHED_NC is None:
        _CACHED_NC = build_nc()
    return _CACHED_NC


def make_in_maps(x, w_qkv, w_out, b_out):
    x = np.ascontiguousarray(x, dtype=np.float32)
    w_qkv = np.asarray(w_qkv, dtype=np.float32)
    w_out = np.asarray(w_out, dtype=np.float32)
    b_out = np.asarray(b_out, dtype=np.float32)
    xf = x.reshape(2, P, N_TOT)

    # full x, fp8, DoubleRowSwInterleave layout: subtile pairs (2t, 2t+1)
    # interleaved per column with columns reversed:
    # [p, (b, t, qq, which)] where element = x^T[subtile 2t+which][p, 127-qq]
    arr = (
        xf.transpose(0, 2, 1)            # (2, n, c)
        .reshape(2, SUB, P, P)           # (2, m, p, c)
    )
    inter = np.stack(
        [arr[:, 0::2, :, ::-1], arr[:, 1::2, :, ::-1]], axis=-1
    )                                    # (2, t, p, qq, which)
    xg_h = np.ascontiguousarray(
        inter.transpose(2, 0, 1, 3, 4).reshape(P, 2 * SUB * P)
    ).astype(f8)

    wpack = np.zeros((P, WCOLS), np.float32)
    wpack[:, 0:512] = w_qkv[:512].T * SCALE
    # rows reversed: the SwInterleave Gram produces G with reversed columns,
    # so a = G' Wq has reversed rows; reversing wk's contraction rows undoes
    # it exactly (G is symmetric).
    wpack[:, 512:1024] = w_qkv[512:1024].T[::-1, :]
    wpack[:, 1024:1536] = (
        (w_qkv[1024:] / N_TOT).reshape(4, P, P).transpose(1, 0, 2).reshape(P, 512)
    )
    for h in range(HEADS):
        wpack[
            64 * (h % 2) : 64 * (h % 2) + 64,
            1536 + 64 * (h // 2) : 1536 + 64 * (h // 2) + 64,
        ] = w_out[:, h * 64 : (h + 1) * 64].T
    wpack[:, 1792] = np.concatenate([b_out, b_out])
    wpack_h = wpack.astype(bf16)

    in_maps = []
    for c in range(NCORES):
        # own output shard, bf16, [c, (b, n)]
        xc_h = np.ascontiguousarray(
            xf[:, :, c * NSH : (c + 1) * NSH].transpose(1, 0, 2).reshape(P, 2 * NSH)
        ).astype(bf16)
        in_maps.append({"xg": xg_h, "xc": xc_h, "wpack": wpack_h})
    return in_maps


def assemble_output(results):
    # out layout: [p = 64*half + row, (b, pair t, 512)]; spatial column of
    # (b, t, half, col) is shard_base + (2t + half)*512 + col.
    y = np.empty((2, 64, N_TOT), np.float32)
    for c in range(NCORES):
        o = np.asarray(results[c]["out"]).astype(np.float32)  # [128, 4096]
        for b in range(2):
            for t in range(4):
                blk = o[:, (b * 4 + t) * 512 : (b * 4 + t + 1) * 512]
                y[b, :, c * NSH + 2 * t * 512 : c * NSH + (2 * t + 1) * 512] = blk[:64]
                y[b, :, c * NSH + (2 * t + 1) * 512 : c * NSH + (2 * t + 2) * 512] = (
                    blk[64:]
                )
    return y.reshape(2, 64, 32, 32, 32)


def kernel(**inputs):
    in_maps = make_in_maps(
        inputs["x"], inputs["w_qkv"], inputs["w_out"], inputs["b_out"]
    )
    nc = _get_nc()
    res = run_bass_kernel_spmd(nc, in_maps, core_ids=list(range(NCORES)))
    return assemble_output(res.results)


# revision 7
# speedup vs baseline: 1.0917x; 1.0917x over previous
"""Trainium2 Bass kernel for nn_Attention (channel-attention, 8 NeuronCores).

Algorithm (algebraically identical to the reference):
  The attention contracts over the spatial axis n = 32*32*32 = 32768, and the
  attention matrices are tiny (64x64 per head).  Everything collapses around
  the per-batch Gram matrix G_b = x_b @ x_b^T (128x128):

    scores_bh = scale * Wq_h G_b Wk_h^T            (tiny)
    attn      = softmax(scores)                     (tiny)
    W_eff_b   = (1/n) * sum_h Wout_h attn_bh Wv_h   (64x128, tiny)
    y_b       = W_eff_b @ x_b + b_out               (the only other big matmul)

  Sharding: NO collectives (an ncfw collective costs 60-80us of firmware
  wakeup on this stack, dwarfing the kernel).  Every core receives the FULL
  x in fp8-e4m3 [n, c] layout (8 MB) and computes the complete Gram
  redundantly (fp8 is harmless: the Gram contracts over 32768 samples), plus
  its own 1/8 spatial shard in bf16 [c, n] layout (2 MB) for the y matmul.

  Performance model (from perfetto/NTFF analysis of the previous version):
  - The input stream is the wall: ~10.9 MB at ~360 GB/s = ~30.5 us.
  - The PE at full clock consumes a fp8-DR Gram pair (256 spatial rows) every
    ~78 ns => 20 us of Gram work, comfortably inside the stream...
  - ...BUT the HW power manager demotes the PE to half clock (HAM k=4/8
    windows in the NTFF) after idle gaps, and re-promotes slowly.  The old
    version starved the PE at chunk boundaries and during softmax waits,
    lost the full clock for ~30 us of the run, and finished the Gram ~13 us
    after the stream ended.
  Fixes here:
  - 256KB-piece-granular streaming (2 pieces per 512 KB chunk) so the PE
    never waits more than ~0.3 us for data.  Pieces stay >=256 KB because
    descriptor issue costs ~0.6 us each on the sync/scalar queues: smaller
    pieces make the stream issue-bound (measured).
  - Zero-data warm matmuls (fp8 DR on a memset tile: no switching power)
    fill the three unavoidable PE gaps: DMA prefill, the b0->b1 stream
    boundary (softmax0 wait), and the softmax1 wait on the tail.
  - The softmax is pipelined per head-group: ACT exp carries bias=-max and
    accum_out=row-sum in ONE instruction, so the chain per group is
    DVE(max) -> ACT(exp+sum) -> DVE(recip, scale) -> PE(mt), overlapped
    across the 4 groups.
  - Phase E packs two 512-col output chunks into one [128, 512] PSUM tile
    (PE quadrant packing), halving the bias-add and output-DMA count.
  - Queue routing: sync+scalar carry only the input stream (gram pieces,
    then xc last so the tail-needed shard arrives exactly at stream end)
    plus the batch-1 outputs (post-stream, FIFO-safe); gpsimd SWDGE carries
    wpack and batch-0 outputs so they never head-of-line block the stream.
"""

import numpy as np
import ml_dtypes

import concourse.bass as bass
import concourse.bacc as bacc
import concourse.mybir as mybir
import concourse.tile as tile
from concourse.bass_utils import run_bass_kernel_spmd

NCORES = 8
P = 128
N_TOT = 32 * 32 * 32          # 32768 spatial points
NSH = N_TOT // NCORES         # 4096 per core per batch (output shard)
SUB = N_TOT // P              # 256 fp8 k-subtiles per batch
CHUNK_SUB = 32                # subtiles per DMA chunk (512 KB)
NCHUNK = SUB // CHUNK_SUB     # 8 chunks per batch
CHW = CHUNK_SUB * P           # 4096 fp8 free columns per chunk
PIECES = 2                    # DMA pieces per chunk (256 KB each)
HEADS = 8
DH = 64
SCALE = DH ** -0.5
WCOLS = 512 + 512 + 512 + 256 + 1  # packed weights: wq|wk|wv|wo|bo
WARM_START = 0                # PE warm-keepers (OFF: the HW throttle is a
WARM_MID = 0                  # utilization budget -- idle EARNS credit, so
WARM_TAIL = 0                 # fillers burn it and stretch the run)
BF = mybir.dt.bfloat16
F32 = mybir.dt.float32
FP8 = mybir.dt.float8e4
DR = mybir.MatmulPerfMode.DoubleRow
DRSW = mybir.MatmulPerfMode.DoubleRowSwInterleave
EXP = mybir.ActivationFunctionType.Exp
bf16 = ml_dtypes.bfloat16
f8 = ml_dtypes.float8_e4m3

_CACHED_NC = None


class _TrimmedTileContext(tile.TileContext):
    """TileContext with a minimal exit sequence.

    The stock exit is drain -> barrier -> sem-clear -> barrier; the
    barrier + clear lower to an EVSEM butterfly measured at ~7us (every
    engine walks the 27-sem global clock).  For a single-shot kernel the
    Sync drain with global-clock waits already gates completion on every
    DMA and engine op, each engine halts in-order after its last
    scheduled instruction, and the engine preamble re-initializes the
    semaphore file on the next execution (verified: back-to-back
    executions of the same loaded NEFF stay correct).  So keep only the
    drain.
    """

    def _drain_and_barrier(self, tick_clock, wait_clock):
        from concourse.vector_clock import ScopedClock

        drain_inst = self.nc.sync.drain()
        wait_clock.add_sem_waits(
            drain_inst.ins, ScopedClock({None: tick_clock.global_clock})
        )
        popped = self.nc._tile_sem_poison_stack.pop()
        assert popped is self._sem_poison


def build_nc():
    # The stock Bass init ends with const-AP memsets guarded by a second
    # all-engine barrier; the consts are unused here and the barrier adds
    # ~2us of start-up serialization, so skip that one barrier only.
    orig_barrier = bass.Bass.all_engine_barrier
    bass.Bass.all_engine_barrier = lambda self: None
    try:
        nc = bacc.Bacc(
            "TRN2", target_bir_lowering=False, debug=False, num_devices=NCORES
        )
    finally:
        bass.Bass.all_engine_barrier = orig_barrier

    # full x, fp8, [p, (b, m, c)] DoubleRow layout: subtile m holds spatial
    # rows m*128..m*128+127 of batch b, channels on the innermost axis.
    xg_ext = nc.dram_tensor("xg", [P, 2 * SUB * P], FP8, kind="ExternalInput")
    # own output shard, bf16, [c, (b, n)] layout for the y matmul
    xc_ext = nc.dram_tensor("xc", [P, 2 * NSH], BF, kind="ExternalInput")
    w_ext = nc.dram_tensor("wpack", [P, WCOLS], BF, kind="ExternalInput")
    # y out, bf16: partition = (chunk-half, row), free = (b, pair, 512)
    out_ext = nc.dram_tensor("out", [P, NSH], BF, kind="ExternalOutput")

    with _TrimmedTileContext(nc) as tc:
        with (
            tc.tile_pool(name="const", bufs=1) as const,
            tc.tile_pool(name="data", bufs=1) as data,
            tc.tile_pool(name="work", bufs=1) as work,
            tc.tile_pool(name="ypool", bufs=8) as ypool,
            tc.tile_pool(name="psg", bufs=2, space="PSUM") as psg,
            tc.tile_pool(name="psd", bufs=2, space="PSUM") as psd,
            tc.tile_pool(name="psy", bufs=4, space="PSUM") as psy,
        ):
            # ---- input DMAs: program order == ring FIFO order ----
            # sync+scalar rings carry ONLY the stream, piece-interleaved:
            # b0 gram, xc0, b1 gram, xc1.  Each 512KB chunk is split into
            # two 256KB pieces on opposite rings so both rings work on the
            # same chunk and the PE's per-piece waits stay ~0.35us.
            xg_tiles = [[], []]
            qs = [nc.sync, nc.scalar]

            def make_xg(b, c):
                t = data.tile([P, CHW], FP8, tag=f"xg{b}_{c}")
                off = (b * SUB + c * CHUNK_SUB) * P
                pw = CHW // PIECES
                for p in range(PIECES):
                    qs[p % 2].dma_start(
                        t[:, p * pw : (p + 1) * pw],
                        xg_ext[:, off + p * pw : off + (p + 1) * pw],
                    )
                xg_tiles[b].append(t)

            xc = data.tile([P, 2 * NSH], BF, tag="xc")
            wpack = const.tile([P, WCOLS], BF, tag="wpack")
            wq = wpack[:, 0:512]
            wk = wpack[:, 512:1024]
            wv = wpack[:, 1024:1536]
            wo = wpack[:, 1536:1792]

            # wpack rides the gpsimd SWDGE ring: needed mid-stream, and it
            # must not displace gram bytes at the head of the hw rings.
            nc.gpsimd.dma_start(wpack[:], w_ext[:])

            # ALL gram first (both batches back to back: no PE famine at the
            # b0->b1 boundary), then xc0, then xc1.  Phase E runs entirely on
            # the tail, where xc arrives exactly when needed and nothing
            # mid-stream ever waits on the descriptor-clogged hw queues.
            for b in range(2):
                for c in range(NCHUNK):
                    make_xg(b, c)
            nc.sync.dma_start(xc[:, 0 : NSH // 2], xc_ext[:, 0 : NSH // 2])
            nc.scalar.dma_start(xc[:, NSH // 2 : NSH], xc_ext[:, NSH // 2 : NSH])
            nc.sync.dma_start(
                xc[:, NSH : NSH + NSH // 2], xc_ext[:, NSH : NSH + NSH // 2]
            )
            nc.scalar.dma_start(
                xc[:, NSH + NSH // 2 :], xc_ext[:, NSH + NSH // 2 :]
            )

            # ---- constants ----
            bo = work.tile([P, 1], F32, tag="bo")
            nc.vector.tensor_copy(bo[:], wpack[:, 1792:1793])

            # ---- Gram accumulation (fp8 DoubleRow) ----
            g_ps = [None, None]
            gbf = [None, None]

            def gram_chunks(b, c_lo, c_hi):
                if g_ps[b] is None:
                    g_ps[b] = psg.tile([P, P], F32, tag="g", name=f"g_ps{b}")
                n_mm = CHUNK_SUB // 2
                for c in range(c_lo, c_hi):
                    xr = xg_tiles[b][c][:].rearrange("p (m q) -> p m q", q=2 * P)
                    for j in range(n_mm):
                        # software-interleaved pair block: per partition the
                        # 256 bytes are [A_c127, B_c127, ..., A_c0, B_c0]
                        # (A/B = the two k-subtiles, columns reversed per the
                        # HW SwInterleave contract).  The weights AP streams
                        # the storage order; the ifmap AP picks plane i at
                        # stride 2.  G comes out with reversed columns,
                        # absorbed by reversing wk's rows host-side.
                        blk = xr[:, j, :]
                        lhsT = blk.rearrange("p (qq two) -> p qq two", two=2)
                        rhs = blk.rearrange("p (qq two) -> p two qq", two=2)
                        nc.tensor.matmul(
                            g_ps[b][:], lhsT, rhs,
                            start=(c == 0 and j == 0),
                            stop=(c == NCHUNK - 1 and j == n_mm - 1),
                            perf_mode=DRSW,
                        )

            # ---- phase D: scores (PE), softmax (DVE/ACT), W_eff (PE) ----
            s_tiles = {}

            def d_scores(b):
                """gbf cast; a = G Wq; S_h = a_h^T Wk_h (quadrant-packed)."""
                gbf[b] = work.tile([P, P], BF, tag=f"gbf{b}", name=f"gbf{b}")
                nc.vector.tensor_copy(gbf[b][:], g_ps[b][:])
                a_ps = psd.tile([P, 512], F32, tag="d", name=f"a_ps{b}")
                a_sb = work.tile([P, 512], BF, tag=f"asb{b}", name=f"a_sb{b}")
                s_ps = psd.tile([P, 256], F32, tag="d", name=f"s_ps{b}")
                nc.tensor.matmul(a_ps[:], gbf[b][:], wq, start=True, stop=True)
                for sl in range(4):
                    nc.vector.tensor_copy(
                        a_sb[:, sl * 128 : (sl + 1) * 128],
                        a_ps[:, sl * 128 : (sl + 1) * 128],
                    )
                for h in range(HEADS):
                    pb = 64 * (h % 2)
                    cg = 64 * (h // 2)
                    nc.tensor.matmul(
                        s_ps[pb : pb + 64, cg : cg + 64],
                        a_sb[:, h * 64 : (h + 1) * 64],
                        wk[:, h * 64 : (h + 1) * 64],
                        start=True, stop=True,
                    )
                s_tiles[b] = s_ps

            def d_softmax(b):
                """Per-group: exp(s - max) with fused row-sum, then scale.

                ACT Exp takes bias = -max (per-partition AP) and emits the
                row sum via accum_out in the same instruction, so the chain
                is DVE(max) -> ACT(exp+sum) -> DVE(recip) -> DVE(scale),
                pipelined across the 4 head-groups.
                """
                s_ps = s_tiles[b]
                negmax = work.tile([P, 4], F32, tag=f"nm{b}", name=f"negmax{b}")
                exp_sb = work.tile([P, 256], F32, tag=f"exp{b}", name=f"exp_sb{b}")
                sums = work.tile([P, 4], F32, tag=f"sums{b}", name=f"sums{b}")
                recip = work.tile([P, 4], F32, tag=f"recip{b}", name=f"recip{b}")
                attn = work.tile([P, 256], BF, tag=f"attn{b}", name=f"attn{b}")
                nc.vector.reduce_max(
                    negmax[:],
                    s_ps[:].rearrange("p (g j) -> p g j", j=64),
                    axis=mybir.AxisListType.X,
                    negate=True,
                )
                for g in range(4):
                    cg = 64 * g
                    nc.scalar.activation(
                        exp_sb[:, cg : cg + 64],
                        s_ps[:, cg : cg + 64],
                        EXP,
                        bias=negmax[:, g : g + 1],
                        scale=1.0,
                        accum_out=sums[:, g : g + 1],
                    )
                    nc.vector.reciprocal(recip[:, g : g + 1], sums[:, g : g + 1])
                    nc.vector.tensor_scalar_mul(
                        attn[:, cg : cg + 64],
                        exp_sb[:, cg : cg + 64],
                        recip[:, g : g + 1],
                    )
                return attn

            def d_weff(b, attn):
                """MT_h = attn_h^T WoT_h; W_eff = wv MT (group-pipelined)."""
                mt_ps = psd.tile([P, 256], F32, tag="d", name=f"mt_ps{b}")
                mt_sb = work.tile([P, 256], BF, tag=f"mt{b}", name=f"mt_sb{b}")
                w_ps = psd.tile([P, 64], F32, tag="d", name=f"w_ps{b}")
                weff = work.tile([P, 64], BF, tag=f"weff{b}", name=f"weff{b}")
                for g in range(4):
                    cg = 64 * g
                    for pb in (0, 64):
                        nc.tensor.matmul(
                            mt_ps[pb : pb + 64, cg : cg + 64],
                            attn[pb : pb + 64, cg : cg + 64],
                            wo[pb : pb + 64, cg : cg + 64],
                            start=True, stop=True,
                        )
                    nc.vector.tensor_copy(
                        mt_sb[:, cg : cg + 64], mt_ps[:, cg : cg + 64]
                    )
                    nc.tensor.matmul(
                        w_ps[:],
                        wv[:, g * P : (g + 1) * P],
                        mt_sb[:, cg : cg + 64],
                        start=(g == 0), stop=(g == 3),
                    )
                nc.vector.tensor_copy(weff[:], w_ps[:])
                return weff

            def phase_e(b, weff, t_lo, t_hi):
                """y_b = W_eff_b @ x_b + b_out, two 512-col chunks per PSUM
                tile via quadrant packing (out partitions 0-63 / 64-127)."""
                for t in range(t_lo, t_hi):
                    y_ps = psy.tile([P, 512], F32, tag="y", name=f"y_ps{b}_{t}")
                    for half in (0, 1):
                        j = 2 * t + half
                        nc.tensor.matmul(
                            y_ps[64 * half : 64 * half + 64, :],
                            weff[:],
                            xc[:, b * NSH + j * 512 : b * NSH + (j + 1) * 512],
                            start=True, stop=True,
                        )
                    y_sb = ypool.tile([P, 512], BF, tag="ysb", name=f"y_sb{b}_{t}")
                    if b == 0:
                        # ACT engine: keeps the DVE free for the D1/E1 chain
                        nc.scalar.activation(
                            y_sb[:], y_ps[:],
                            mybir.ActivationFunctionType.Identity,
                            bias=bo[:, 0:1], scale=1.0,
                        )
                        nc.gpsimd.dma_start(
                            out_ext[:, (b * 4 + t) * 512 : (b * 4 + t + 1) * 512],
                            y_sb[:],
                        )
                    else:
                        nc.vector.tensor_scalar_add(y_sb[:], y_ps[:], bo[:, 0:1])
                        qs[t % 2].dma_start(
                            out_ext[:, (b * 4 + t) * 512 : (b * 4 + t + 1) * 512],
                            y_sb[:],
                        )

            # ---- PE program order ----
            # gram0 -> gram1 back to back (stream-paced, no boundary gap:
            # D0's scores slot in after gram1's first chunk so the gbf0 cast
            # latency hides under data-paced matmuls).  The whole back half
            # (weff0/E0/weff1/E1) runs on the tail: weff0+E0 cover the
            # softmax1 ACT/DVE chain, and xc0/xc1 arrive (in that order)
            # right as phase E consumes them.
            gram_chunks(0, 0, NCHUNK)
            gram_chunks(1, 0, 1)
            d_scores(0)
            attn0 = d_softmax(0)
            gram_chunks(1, 1, NCHUNK)
            d_scores(1)
            attn1 = d_softmax(1)
            weff0 = d_weff(0, attn0)
            phase_e(0, weff0, 0, 4)
            weff1 = d_weff(1, attn1)
            phase_e(1, weff1, 0, 4)

    nc.compile()
    return nc


def _get_nc():
    global _CACHED_NC
    if _CACHED_NC is None:
        _CACHED_NC = build_nc()
    return _CACHED_NC


def make_in_maps(x, w_qkv, w_out, b_out):
    x = np.ascontiguousarray(x, dtype=np.float32)
    w_qkv = np.asarray(w_qkv, dtype=np.float32)
    w_out = np.asarray(w_out, dtype=np.float32)
    b_out = np.asarray(b_out, dtype=np.float32)
    xf = x.reshape(2, P, N_TOT)

    # full x, fp8, DoubleRowSwInterleave layout: subtile pairs (2t, 2t+1)
    # interleaved per column with columns reversed:
    # [p, (b, t, qq, which)] where element = x^T[subtile 2t+which][p, 127-qq]
    arr = (
        xf.transpose(0, 2, 1)            # (2, n, c)
        .reshape(2, SUB, P, P)           # (2, m, p, c)
    )
    inter = np.stack(
        [arr[:, 0::2, :, ::-1], arr[:, 1::2, :, ::-1]], axis=-1
    )                                    # (2, t, p, qq, which)
    xg_h = np.ascontiguousarray(
        inter.transpose(2, 0, 1, 3, 4).reshape(P, 2 * SUB * P)
    ).astype(f8)

    wpack = np.zeros((P, WCOLS), np.float32)
    wpack[:, 0:512] = w_qkv[:512].T * SCALE
    # rows reversed: the SwInterleave Gram produces G with reversed columns,
    # so a = G' Wq has reversed rows; reversing wk's contraction rows undoes
    # it exactly (G is symmetric).
    wpack[:, 512:1024] = w_qkv[512:1024].T[::-1, :]
    wpack[:, 1024:1536] = (
        (w_qkv[1024:] / N_TOT).reshape(4, P, P).transpose(1, 0, 2).reshape(P, 512)
    )
    for h in range(HEADS):
        wpack[
            64 * (h % 2) : 64 * (h % 2) + 64,
            1536 + 64 * (h // 2) : 1536 + 64 * (h // 2) + 64,
        ] = w_out[:, h * 64 : (h + 1) * 64].T
    wpack[:, 1792] = np.concatenate([b_out, b_out])
    wpack_h = wpack.astype(bf16)

    in_maps = []
    for c in range(NCORES):
        # own output shard, bf16, [c, (b, n)]
        xc_h = np.ascontiguousarray(
            xf[:, :, c * NSH : (c + 1) * NSH].transpose(1, 0, 2).reshape(P, 2 * NSH)
        ).astype(bf16)
        in_maps.append({"xg": xg_h, "xc": xc_h, "wpack": wpack_h})
    return in_maps


def assemble_output(results):
    # out layout: [p = 64*half + row, (b, pair t, 512)]; spatial column of
    # (b, t, half, col) is shard_base + (2t + half)*512 + col.
    y = np.empty((2, 64, N_TOT), np.float32)
    for c in range(NCORES):
        o = np.asarray(results[c]["out"]).astype(np.float32)  # [128, 4096]
        for b in range(2):
            for t in range(4):
                blk = o[:, (b * 4 + t) * 512 : (b * 4 + t + 1) * 512]
                y[b, :, c * NSH + 2 * t * 512 : c * NSH + (2 * t + 1) * 512] = blk[:64]
                y[b, :, c * NSH + (2 * t + 1) * 512 : c * NSH + (2 * t + 2) * 512] = (
                    blk[64:]
                )
    return y.reshape(2, 64, 32, 32, 32)


def kernel(**inputs):
    in_maps = make_in_maps(
        inputs["x"], inputs["w_qkv"], inputs["w_out"], inputs["b_out"]
    )
    nc = _get_nc()
    res = run_bass_kernel_spmd(nc, in_maps, core_ids=list(range(NCORES)))
    return assemble_output(res.results)


# revision 8
# speedup vs baseline: 1.1359x; 1.0405x over previous
"""Trainium2 Bass kernel for nn_Attention (channel-attention, 8 NeuronCores).

Algorithm (algebraically identical to the reference):
  The attention contracts over the spatial axis n = 32*32*32 = 32768, and the
  attention matrices are tiny (64x64 per head).  Everything collapses around
  the per-batch Gram matrix G_b = x_b @ x_b^T (128x128):

    scores_bh = scale * Wq_h G_b Wk_h^T            (tiny)
    attn      = softmax(scores)                     (tiny)
    W_eff_b   = (1/n) * sum_h Wout_h attn_bh Wv_h   (64x128, tiny)
    y_b       = W_eff_b @ x_b + b_out               (the only other big matmul)

  Sharding: NO collectives (an ncfw collective costs 60-80us of firmware
  wakeup on this stack, dwarfing the kernel).  Every core receives the FULL
  x in fp8-e4m3 [n, c] layout (8 MB) and computes the complete Gram
  redundantly (fp8 is harmless: the Gram contracts over 32768 samples), plus
  its own 1/8 spatial shard in bf16 [c, n] layout (2 MB) for the y matmul.

  Performance model (from perfetto/NTFF analysis of the previous version):
  - The input stream is the wall: ~10.9 MB at ~360 GB/s = ~30.5 us.
  - The PE at full clock consumes a fp8-DR Gram pair (256 spatial rows) every
    ~78 ns => 20 us of Gram work, comfortably inside the stream...
  - ...BUT the HW power manager demotes the PE to half clock (HAM k=4/8
    windows in the NTFF) after idle gaps, and re-promotes slowly.  The old
    version starved the PE at chunk boundaries and during softmax waits,
    lost the full clock for ~30 us of the run, and finished the Gram ~13 us
    after the stream ended.
  Fixes here:
  - 256KB-piece-granular streaming (2 pieces per 512 KB chunk) so the PE
    never waits more than ~0.3 us for data.  Pieces stay >=256 KB because
    descriptor issue costs ~0.6 us each on the sync/scalar queues: smaller
    pieces make the stream issue-bound (measured).
  - Zero-data warm matmuls (fp8 DR on a memset tile: no switching power)
    fill the three unavoidable PE gaps: DMA prefill, the b0->b1 stream
    boundary (softmax0 wait), and the softmax1 wait on the tail.
  - The softmax is pipelined per head-group: ACT exp carries bias=-max and
    accum_out=row-sum in ONE instruction, so the chain per group is
    DVE(max) -> ACT(exp+sum) -> DVE(recip, scale) -> PE(mt), overlapped
    across the 4 groups.
  - Phase E packs two 512-col output chunks into one [128, 512] PSUM tile
    (PE quadrant packing), halving the bias-add and output-DMA count.
  - Queue routing: sync+scalar carry only the input stream (gram pieces,
    then xc last so the tail-needed shard arrives exactly at stream end)
    plus the batch-1 outputs (post-stream, FIFO-safe); gpsimd SWDGE carries
    wpack and batch-0 outputs so they never head-of-line block the stream.
"""

import numpy as np
import ml_dtypes

import concourse.bass as bass
import concourse.bacc as bacc
import concourse.mybir as mybir
import concourse.tile as tile
from concourse.bass_utils import run_bass_kernel_spmd

NCORES = 8
P = 128
N_TOT = 32 * 32 * 32          # 32768 spatial points
NSH = N_TOT // NCORES         # 4096 per core per batch (output shard)
SUB = N_TOT // P              # 256 fp8 k-subtiles per batch
CHUNK_SUB = 32                # subtiles per DMA chunk (512 KB)
NCHUNK = SUB // CHUNK_SUB     # 8 chunks per batch
CHW = CHUNK_SUB * P           # 4096 fp8 free columns per chunk
PIECES = 2                    # DMA pieces per chunk (256 KB each)
HEADS = 8
DH = 64
SCALE = DH ** -0.5
WCOLS = 512 + 512 + 512 + 256 + 1  # packed weights: wq|wk|wv|wo|bo
WARM_START = 0                # PE warm-keepers (OFF: the HW throttle is a
WARM_MID = 0                  # utilization budget -- idle EARNS credit, so
WARM_TAIL = 0                 # fillers burn it and stretch the run)
BF = mybir.dt.bfloat16
F32 = mybir.dt.float32
FP8 = mybir.dt.float8e4
DR = mybir.MatmulPerfMode.DoubleRow
DRSW = mybir.MatmulPerfMode.DoubleRowSwInterleave
EXP = mybir.ActivationFunctionType.Exp
bf16 = ml_dtypes.bfloat16
f8 = ml_dtypes.float8_e4m3

_CACHED_NC = None


class _TrimmedTileContext(tile.TileContext):
    """TileContext with a minimal exit sequence.

    The stock exit is drain -> barrier -> sem-clear -> barrier; the
    barrier + clear lower to an EVSEM butterfly measured at ~7us (every
    engine walks the 27-sem global clock).  For a single-shot kernel the
    Sync drain with global-clock waits already gates completion on every
    DMA and engine op, each engine halts in-order after its last
    scheduled instruction, and the engine preamble re-initializes the
    semaphore file on the next execution (verified: back-to-back
    executions of the same loaded NEFF stay correct).  So keep only the
    drain.
    """

    def _drain_and_barrier(self, tick_clock, wait_clock):
        from concourse.vector_clock import ScopedClock

        drain_inst = self.nc.sync.drain()
        wait_clock.add_sem_waits(
            drain_inst.ins, ScopedClock({None: tick_clock.global_clock})
        )
        popped = self.nc._tile_sem_poison_stack.pop()
        assert popped is self._sem_poison


def build_nc():
    # The stock Bass init ends with const-AP memsets guarded by a second
    # all-engine barrier; the consts are unused here and the barrier adds
    # ~2us of start-up serialization, so skip that one barrier only.
    orig_barrier = bass.Bass.all_engine_barrier
    bass.Bass.all_engine_barrier = lambda self: None
    try:
        nc = bacc.Bacc(
            "TRN2", target_bir_lowering=False, debug=False, num_devices=NCORES
        )
    finally:
        bass.Bass.all_engine_barrier = orig_barrier

    # full x, fp8, [p, (b, m, c)] DoubleRow layout: subtile m holds spatial
    # rows m*128..m*128+127 of batch b, channels on the innermost axis.
    xg_ext = nc.dram_tensor("xg", [P, 2 * SUB * P], FP8, kind="ExternalInput")
    # own output shard, bf16, [c, (b, n)] layout for the y matmul
    xc_ext = nc.dram_tensor("xc", [P, 2 * NSH], BF, kind="ExternalInput")
    w_ext = nc.dram_tensor("wpack", [P, WCOLS], BF, kind="ExternalInput")
    # y out, bf16: partition = (chunk-half, row), free = (b, pair, 512)
    out_ext = nc.dram_tensor("out", [P, NSH], BF, kind="ExternalOutput")

    with _TrimmedTileContext(nc) as tc:
        with (
            tc.tile_pool(name="const", bufs=1) as const,
            tc.tile_pool(name="data", bufs=1) as data,
            tc.tile_pool(name="work", bufs=1) as work,
            tc.tile_pool(name="ypool", bufs=8) as ypool,
            tc.tile_pool(name="psg", bufs=2, space="PSUM") as psg,
            tc.tile_pool(name="psd", bufs=2, space="PSUM") as psd,
            tc.tile_pool(name="psy", bufs=4, space="PSUM") as psy,
        ):
            # ---- input DMAs: program order == ring FIFO order ----
            # sync+scalar rings carry ONLY the stream, piece-interleaved:
            # b0 gram, xc0, b1 gram, xc1.  Each 512KB chunk is split into
            # two 256KB pieces on opposite rings so both rings work on the
            # same chunk and the PE's per-piece waits stay ~0.35us.
            xg_tiles = [[], []]
            qs = [nc.sync, nc.scalar]

            def make_xg(b, c):
                t = data.tile([P, CHW], FP8, tag=f"xg{b}_{c}")
                off = (b * SUB + c * CHUNK_SUB) * P
                pw = CHW // PIECES
                for p in range(PIECES):
                    qs[p % 2].dma_start(
                        t[:, p * pw : (p + 1) * pw],
                        xg_ext[:, off + p * pw : off + (p + 1) * pw],
                    )
                xg_tiles[b].append(t)

            xc = data.tile([P, 2 * NSH], BF, tag="xc")
            wpack = const.tile([P, WCOLS], BF, tag="wpack")
            wq = wpack[:, 0:512]
            wk = wpack[:, 512:1024]
            wv = wpack[:, 1024:1536]
            wo = wpack[:, 1536:1792]

            # wpack rides the gpsimd SWDGE ring: needed mid-stream, and it
            # must not displace gram bytes at the head of the hw rings.
            nc.gpsimd.dma_start(wpack[:], w_ext[:])

            # ALL gram first (both batches back to back: no PE famine at the
            # b0->b1 boundary), then xc0, then xc1.  Phase E runs entirely on
            # the tail, where xc arrives exactly when needed and nothing
            # mid-stream ever waits on the descriptor-clogged hw queues.
            for b in range(2):
                for c in range(NCHUNK):
                    make_xg(b, c)
            nc.sync.dma_start(xc[:, 0 : NSH // 2], xc_ext[:, 0 : NSH // 2])
            nc.scalar.dma_start(xc[:, NSH // 2 : NSH], xc_ext[:, NSH // 2 : NSH])
            nc.sync.dma_start(
                xc[:, NSH : NSH + NSH // 2], xc_ext[:, NSH : NSH + NSH // 2]
            )
            nc.scalar.dma_start(
                xc[:, NSH + NSH // 2 :], xc_ext[:, NSH + NSH // 2 :]
            )

            # ---- constants ----
            bo = work.tile([P, 1], F32, tag="bo")
            nc.vector.tensor_copy(bo[:], wpack[:, 1792:1793])

            # ---- Gram accumulation (fp8 DoubleRow) ----
            g_ps = [None, None]
            gbf = [None, None]

            def gram_chunks(b, c_lo, c_hi):
                if g_ps[b] is None:
                    g_ps[b] = psg.tile([P, P], F32, tag="g", name=f"g_ps{b}")
                n_mm = CHUNK_SUB // 2
                for c in range(c_lo, c_hi):
                    xr = xg_tiles[b][c][:].rearrange("p (m q) -> p m q", q=2 * P)
                    for j in range(n_mm):
                        # software-interleaved pair block: per partition the
                        # 256 bytes are [A_c127, B_c127, ..., A_c0, B_c0]
                        # (A/B = the two k-subtiles, columns reversed per the
                        # HW SwInterleave contract).  The weights AP streams
                        # the storage order; the ifmap AP picks plane i at
                        # stride 2.  G comes out with reversed columns,
                        # absorbed by reversing wk's rows host-side.
                        blk = xr[:, j, :]
                        lhsT = blk.rearrange("p (qq two) -> p qq two", two=2)
                        rhs = blk.rearrange("p (qq two) -> p two qq", two=2)
                        nc.tensor.matmul(
                            g_ps[b][:], lhsT, rhs,
                            start=(c == 0 and j == 0),
                            stop=(c == NCHUNK - 1 and j == n_mm - 1),
                            perf_mode=DRSW,
                        )

            # ---- phase D: scores (PE), softmax (DVE/ACT), W_eff (PE) ----
            s_tiles = {}

            def d_scores(b):
                """gbf cast; a = G Wq; S_h = a_h^T Wk_h (quadrant-packed)."""
                gbf[b] = work.tile([P, P], BF, tag=f"gbf{b}", name=f"gbf{b}")
                nc.vector.tensor_copy(gbf[b][:], g_ps[b][:])
                a_ps = psd.tile([P, 512], F32, tag="d", name=f"a_ps{b}")
                a_sb = work.tile([P, 512], BF, tag=f"asb{b}", name=f"a_sb{b}")
                s_ps = psd.tile([P, 256], F32, tag="d", name=f"s_ps{b}")
                nc.tensor.matmul(a_ps[:], gbf[b][:], wq, start=True, stop=True)
                nc.vector.tensor_copy(a_sb[:], a_ps[:])
                for h in range(HEADS):
                    pb = 64 * (h % 2)
                    cg = 64 * (h // 2)
                    nc.tensor.matmul(
                        s_ps[pb : pb + 64, cg : cg + 64],
                        a_sb[:, h * 64 : (h + 1) * 64],
                        wk[:, h * 64 : (h + 1) * 64],
                        start=True, stop=True,
                    )
                s_tiles[b] = s_ps

            def d_softmax(b):
                """Per-group: exp(s - max) with fused row-sum, then scale.

                ACT Exp takes bias = -max (per-partition AP) and emits the
                row sum via accum_out in the same instruction, so the chain
                is DVE(max) -> ACT(exp+sum) -> DVE(recip) -> DVE(scale),
                pipelined across the 4 head-groups.
                """
                s_ps = s_tiles[b]
                negmax = work.tile([P, 4], F32, tag=f"nm{b}", name=f"negmax{b}")
                exp_sb = work.tile([P, 256], F32, tag=f"exp{b}", name=f"exp_sb{b}")
                sums = work.tile([P, 4], F32, tag=f"sums{b}", name=f"sums{b}")
                recip = work.tile([P, 4], F32, tag=f"recip{b}", name=f"recip{b}")
                attn = work.tile([P, 256], BF, tag=f"attn{b}", name=f"attn{b}")
                nc.vector.reduce_max(
                    negmax[:],
                    s_ps[:].rearrange("p (g j) -> p g j", j=64),
                    axis=mybir.AxisListType.X,
                    negate=True,
                )
                for g in range(4):
                    cg = 64 * g
                    nc.scalar.activation(
                        exp_sb[:, cg : cg + 64],
                        s_ps[:, cg : cg + 64],
                        EXP,
                        bias=negmax[:, g : g + 1],
                        scale=1.0,
                        accum_out=sums[:, g : g + 1],
                    )
                    nc.vector.reciprocal(recip[:, g : g + 1], sums[:, g : g + 1])
                    nc.vector.tensor_scalar_mul(
                        attn[:, cg : cg + 64],
                        exp_sb[:, cg : cg + 64],
                        recip[:, g : g + 1],
                    )
                return attn

            def d_weff(b, attn):
                """MT_h = attn_h^T WoT_h; W_eff = wv MT.

                Monolithic: 8 quadrant matmuls, ONE [128,256] cast, 4
                accumulating matmuls, one weff cast -- two cross-engine
                hops total.  (A per-group cast<->matmul ping-pong costs 8
                hops and serializes ~4us on the tail.)
                """
                mt_ps = psd.tile([P, 256], F32, tag="d", name=f"mt_ps{b}")
                mt_sb = work.tile([P, 256], BF, tag=f"mt{b}", name=f"mt_sb{b}")
                w_ps = psd.tile([P, 64], F32, tag="d", name=f"w_ps{b}")
                weff = work.tile([P, 64], BF, tag=f"weff{b}", name=f"weff{b}")
                for h in range(HEADS):
                    pb = 64 * (h % 2)
                    cg = 64 * (h // 2)
                    nc.tensor.matmul(
                        mt_ps[pb : pb + 64, cg : cg + 64],
                        attn[pb : pb + 64, cg : cg + 64],
                        wo[pb : pb + 64, cg : cg + 64],
                        start=True, stop=True,
                    )
                nc.vector.tensor_copy(mt_sb[:], mt_ps[:])
                for g in range(4):
                    nc.tensor.matmul(
                        w_ps[:],
                        wv[:, g * P : (g + 1) * P],
                        mt_sb[:, g * 64 : (g + 1) * 64],
                        start=(g == 0), stop=(g == 3),
                    )
                nc.vector.tensor_copy(weff[:], w_ps[:])
                return weff

            def phase_e(b, weff, t_lo, t_hi):
                """y_b = W_eff_b @ x_b + b_out, two 512-col chunks per PSUM
                tile via quadrant packing (out partitions 0-63 / 64-127)."""
                for t in range(t_lo, t_hi):
                    y_ps = psy.tile([P, 512], F32, tag="y", name=f"y_ps{b}_{t}")
                    for half in (0, 1):
                        j = 2 * t + half
                        nc.tensor.matmul(
                            y_ps[64 * half : 64 * half + 64, :],
                            weff[:],
                            xc[:, b * NSH + j * 512 : b * NSH + (j + 1) * 512],
                            start=True, stop=True,
                        )
                    y_sb = ypool.tile([P, 512], BF, tag="ysb", name=f"y_sb{b}_{t}")
                    # alternate ACT/DVE so consecutive pairs' bias-adds run
                    # in parallel instead of serializing on one engine
                    if t % 2 == 0:
                        nc.scalar.activation(
                            y_sb[:], y_ps[:],
                            mybir.ActivationFunctionType.Identity,
                            bias=bo[:, 0:1], scale=1.0,
                        )
                    else:
                        nc.vector.tensor_scalar_add(y_sb[:], y_ps[:], bo[:, 0:1])
                    dst = out_ext[:, (b * 4 + t) * 512 : (b * 4 + t + 1) * 512]
                    if b == 0:
                        nc.gpsimd.dma_start(dst, y_sb[:])
                    else:
                        qs[t % 2].dma_start(dst, y_sb[:])

            # ---- PE program order ----
            # gram0 -> gram1 back to back (stream-paced, no boundary gap:
            # D0's scores slot in after gram1's first chunk so the gbf0 cast
            # latency hides under data-paced matmuls).  The whole back half
            # (weff0/E0/weff1/E1) runs on the tail: weff0+E0 cover the
            # softmax1 ACT/DVE chain, and xc0/xc1 arrive (in that order)
            # right as phase E consumes them.
            gram_chunks(0, 0, NCHUNK)
            gram_chunks(1, 0, 1)
            d_scores(0)
            attn0 = d_softmax(0)
            gram_chunks(1, 1, NCHUNK - 1)
            weff0 = d_weff(0, attn0)   # fills the last chunk's data wait
            gram_chunks(1, NCHUNK - 1, NCHUNK)
            d_scores(1)
            attn1 = d_softmax(1)
            phase_e(0, weff0, 0, 4)    # covers the softmax1 ACT/DVE chain
            weff1 = d_weff(1, attn1)
            phase_e(1, weff1, 0, 4)

    nc.compile()
    return nc


def _get_nc():
    global _CACHED_NC
    if _CACHED_NC is None:
        _CACHED_NC = build_nc()
    return _CACHED_NC


def make_in_maps(x, w_qkv, w_out, b_out):
    x = np.ascontiguousarray(x, dtype=np.float32)
    w_qkv = np.asarray(w_qkv, dtype=np.float32)
    w_out = np.asarray(w_out, dtype=np.float32)
    b_out = np.asarray(b_out, dtype=np.float32)
    xf = x.reshape(2, P, N_TOT)

    # full x, fp8, DoubleRowSwInterleave layout: subtile pairs (2t, 2t+1)
    # interleaved per column with columns reversed:
    # [p, (b, t, qq, which)] where element = x^T[subtile 2t+which][p, 127-qq]
    arr = (
        xf.transpose(0, 2, 1)            # (2, n, c)
        .reshape(2, SUB, P, P)           # (2, m, p, c)
    )
    inter = np.stack(
        [arr[:, 0::2, :, ::-1], arr[:, 1::2, :, ::-1]], axis=-1
    )                                    # (2, t, p, qq, which)
    xg_h = np.ascontiguousarray(
        inter.transpose(2, 0, 1, 3, 4).reshape(P, 2 * SUB * P)
    ).astype(f8)

    wpack = np.zeros((P, WCOLS), np.float32)
    wpack[:, 0:512] = w_qkv[:512].T * SCALE
    # rows reversed: the SwInterleave Gram produces G with reversed columns,
    # so a = G' Wq has reversed rows; reversing wk's contraction rows undoes
    # it exactly (G is symmetric).
    wpack[:, 512:1024] = w_qkv[512:1024].T[::-1, :]
    wpack[:, 1024:1536] = (
        (w_qkv[1024:] / N_TOT).reshape(4, P, P).transpose(1, 0, 2).reshape(P, 512)
    )
    for h in range(HEADS):
        wpack[
            64 * (h % 2) : 64 * (h % 2) + 64,
            1536 + 64 * (h // 2) : 1536 + 64 * (h // 2) + 64,
        ] = w_out[:, h * 64 : (h + 1) * 64].T
    wpack[:, 1792] = np.concatenate([b_out, b_out])
    wpack_h = wpack.astype(bf16)

    in_maps = []
    for c in range(NCORES):
        # own output shard, bf16, [c, (b, n)]
        xc_h = np.ascontiguousarray(
            xf[:, :, c * NSH : (c + 1) * NSH].transpose(1, 0, 2).reshape(P, 2 * NSH)
        ).astype(bf16)
        in_maps.append({"xg": xg_h, "xc": xc_h, "wpack": wpack_h})
    return in_maps


def assemble_output(results):
    # out layout: [p = 64*half + row, (b, pair t, 512)]; spatial column of
    # (b, t, half, col) is shard_base + (2t + half)*512 + col.
    y = np.empty((2, 64, N_TOT), np.float32)
    for c in range(NCORES):
        o = np.asarray(results[c]["out"]).astype(np.float32)  # [128, 4096]
        for b in range(2):
            for t in range(4):
                blk = o[:, (b * 4 + t) * 512 : (b * 4 + t + 1) * 512]
                y[b, :, c * NSH + 2 * t * 512 : c * NSH + (2 * t + 1) * 512] = blk[:64]
                y[b, :, c * NSH + (2 * t + 1) * 512 : c * NSH + (2 * t + 2) * 512] = (
                    blk[64:]
                )
    return y.reshape(2, 64, 32, 32, 32)


def kernel(**inputs):
    in_maps = make_in_maps(
        inputs["x"], inputs["w_qkv"], inputs["w_out"], inputs["b_out"]
    )
    nc = _get_nc()
    res = run_bass_kernel_spmd(nc, in_maps, core_ids=list(range(NCORES)))
    return assemble_output(res.results)


# revision 9
# speedup vs baseline: 1.2493x; 1.0998x over previous
"""Trainium2 Bass kernel for nn_Attention (channel-attention, 8 NeuronCores).

Algorithm (algebraically identical to the reference):
  The attention contracts over the spatial axis n = 32*32*32 = 32768, and the
  attention matrices are tiny (64x64 per head).  Everything collapses around
  the per-batch Gram matrix G_b = x_b @ x_b^T (128x128):

    scores_bh = scale * Wq_h G_b Wk_h^T            (tiny)
    attn      = softmax(scores)                     (tiny)
    W_eff_b   = (1/n) * sum_h Wout_h attn_bh Wv_h   (64x128, tiny)
    y_b       = W_eff_b @ x_b + b_out               (the only other big matmul)

  Sharding: NO collectives (an ncfw collective costs 60-80us of firmware
  wakeup on this stack, dwarfing the kernel).  Every core receives the FULL
  x in fp8-e4m3 [n, c] layout (8 MB) and computes the complete Gram
  redundantly (fp8 is harmless: the Gram contracts over 32768 samples), plus
  its own 1/8 spatial shard in bf16 [c, n] layout (2 MB) for the y matmul.

  Performance model (from perfetto/NTFF analysis of the previous version):
  - The input stream is the wall: ~10.9 MB at ~360 GB/s = ~30.5 us.
  - The PE at full clock consumes a fp8-DR Gram pair (256 spatial rows) every
    ~78 ns => 20 us of Gram work, comfortably inside the stream...
  - ...BUT the HW power manager demotes the PE to half clock (HAM k=4/8
    windows in the NTFF) after idle gaps, and re-promotes slowly.  The old
    version starved the PE at chunk boundaries and during softmax waits,
    lost the full clock for ~30 us of the run, and finished the Gram ~13 us
    after the stream ended.
  Fixes here:
  - 256KB-piece-granular streaming (2 pieces per 512 KB chunk) so the PE
    never waits more than ~0.3 us for data.  Pieces stay >=256 KB because
    descriptor issue costs ~0.6 us each on the sync/scalar queues: smaller
    pieces make the stream issue-bound (measured).
  - Zero-data warm matmuls (fp8 DR on a memset tile: no switching power)
    fill the three unavoidable PE gaps: DMA prefill, the b0->b1 stream
    boundary (softmax0 wait), and the softmax1 wait on the tail.
  - The softmax is pipelined per head-group: ACT exp carries bias=-max and
    accum_out=row-sum in ONE instruction, so the chain per group is
    DVE(max) -> ACT(exp+sum) -> DVE(recip, scale) -> PE(mt), overlapped
    across the 4 groups.
  - Phase E packs two 512-col output chunks into one [128, 512] PSUM tile
    (PE quadrant packing), halving the bias-add and output-DMA count.
  - Queue routing: sync+scalar carry only the input stream (gram pieces,
    then xc last so the tail-needed shard arrives exactly at stream end)
    plus the batch-1 outputs (post-stream, FIFO-safe); gpsimd SWDGE carries
    wpack and batch-0 outputs so they never head-of-line block the stream.
"""

import numpy as np
import ml_dtypes

import concourse.bass as bass
import concourse.bacc as bacc
import concourse.mybir as mybir
import concourse.tile as tile
from concourse.bass_utils import run_bass_kernel_spmd

NCORES = 8
P = 128
N_TOT = 32 * 32 * 32          # 32768 spatial points
NSH = N_TOT // NCORES         # 4096 per core per batch (output shard)
SUB = N_TOT // P              # 256 fp8 k-subtiles per batch
CHUNK_SUB = 32                # subtiles per DMA chunk (512 KB)
NCHUNK = SUB // CHUNK_SUB     # 8 chunks per batch
CHW = CHUNK_SUB * P           # 4096 fp8 free columns per chunk
PIECES = 2                    # DMA pieces per chunk (256 KB each)
HEADS = 8
DH = 64
SCALE = DH ** -0.5
WCOLS = 512 + 512 + 512 + 256 + 1  # packed weights: wq|wk|wv|wo|bo
WARM_START = 0                # PE warm-keepers (OFF: the HW throttle is a
WARM_MID = 0                  # utilization budget -- idle EARNS credit, so
WARM_TAIL = 0                 # fillers burn it and stretch the run)
BF = mybir.dt.bfloat16
F32 = mybir.dt.float32
FP8 = mybir.dt.float8e4
DR = mybir.MatmulPerfMode.DoubleRow
DRSW = mybir.MatmulPerfMode.DoubleRowSwInterleave
EXP = mybir.ActivationFunctionType.Exp
bf16 = ml_dtypes.bfloat16
f8 = ml_dtypes.float8_e4m3

_CACHED_NC = None


class _TrimmedTileContext(tile.TileContext):
    """TileContext with a minimal exit sequence.

    The stock exit is drain -> barrier -> sem-clear -> barrier; the
    barrier + clear lower to an EVSEM butterfly measured at ~7us (every
    engine walks the 27-sem global clock).  For a single-shot kernel the
    Sync drain with global-clock waits already gates completion on every
    DMA and engine op, each engine halts in-order after its last
    scheduled instruction, and the engine preamble re-initializes the
    semaphore file on the next execution (verified: back-to-back
    executions of the same loaded NEFF stay correct).  So keep only the
    drain.
    """

    def _drain_and_barrier(self, tick_clock, wait_clock):
        from concourse.vector_clock import ScopedClock

        drain_inst = self.nc.sync.drain()
        wait_clock.add_sem_waits(
            drain_inst.ins, ScopedClock({None: tick_clock.global_clock})
        )
        popped = self.nc._tile_sem_poison_stack.pop()
        assert popped is self._sem_poison


def build_nc():
    # The stock Bass init ends with const-AP memsets guarded by a second
    # all-engine barrier; the consts are unused here and the barrier adds
    # ~2us of start-up serialization, so skip that one barrier only.
    orig_barrier = bass.Bass.all_engine_barrier
    bass.Bass.all_engine_barrier = lambda self: None
    try:
        nc = bacc.Bacc(
            "TRN2", target_bir_lowering=False, debug=False, num_devices=NCORES
        )
    finally:
        bass.Bass.all_engine_barrier = orig_barrier

    # full x, fp8, [p, (b, m, c)] DoubleRow layout: subtile m holds spatial
    # rows m*128..m*128+127 of batch b, channels on the innermost axis.
    xg_ext = nc.dram_tensor("xg", [P, 2 * SUB * P], FP8, kind="ExternalInput")
    # own output shard, bf16, [c, (b, n)] layout for the y matmul
    xc_ext = nc.dram_tensor("xc", [P, 2 * NSH], BF, kind="ExternalInput")
    w_ext = nc.dram_tensor("wpack", [P, WCOLS], BF, kind="ExternalInput")
    # y out, bf16: partition = (chunk-half, row), free = (b, pair, 512)
    out_ext = nc.dram_tensor("out", [P, NSH], BF, kind="ExternalOutput")

    with _TrimmedTileContext(nc) as tc:
        with (
            tc.tile_pool(name="const", bufs=1) as const,
            tc.tile_pool(name="data", bufs=1) as data,
            tc.tile_pool(name="work", bufs=1) as work,
            tc.tile_pool(name="ypool", bufs=8) as ypool,
            tc.tile_pool(name="psg", bufs=2, space="PSUM") as psg,
            tc.tile_pool(name="psd", bufs=2, space="PSUM") as psd,
            tc.tile_pool(name="psy", bufs=4, space="PSUM") as psy,
        ):
            # ---- input DMAs: program order == ring FIFO order ----
            # sync+scalar rings carry ONLY the stream, piece-interleaved:
            # b0 gram, xc0, b1 gram, xc1.  Each 512KB chunk is split into
            # two 256KB pieces on opposite rings so both rings work on the
            # same chunk and the PE's per-piece waits stay ~0.35us.
            xg_tiles = [[], []]
            qs = [nc.sync, nc.scalar]
            # ALL input descriptors ride the sync ring alone (one HWDGE ring
            # stripes across all DMA engines, and 256KB/issue keeps issue
            # capacity ~427GB/s above the ~360GB/s stream).  This keeps the
            # scalar queue descriptor-free: its ACT exps would otherwise sit
            # behind ring-backpressured issues until the whole stream had
            # been enqueued (~34us), stalling the batch-0 attention chain.
            inq = [nc.sync, nc.sync]

            def make_xg(b, c):
                t = data.tile([P, CHW], FP8, tag=f"xg{b}_{c}")
                off = (b * SUB + c * CHUNK_SUB) * P
                pw = CHW // PIECES
                for p in range(PIECES):
                    inq[p % 2].dma_start(
                        t[:, p * pw : (p + 1) * pw],
                        xg_ext[:, off + p * pw : off + (p + 1) * pw],
                    )
                xg_tiles[b].append(t)

            xc = data.tile([P, 2 * NSH], BF, tag="xc")
            wpack = const.tile([P, WCOLS], BF, tag="wpack")
            wq = wpack[:, 0:512]
            wk = wpack[:, 512:1024]
            wv = wpack[:, 1024:1536]
            wo = wpack[:, 1536:1792]

            # wpack rides the gpsimd SWDGE ring: needed mid-stream, and it
            # must not displace gram bytes at the head of the hw rings.
            nc.gpsimd.dma_start(wpack[:], w_ext[:])

            # ALL gram first (both batches back to back: no PE famine at the
            # b0->b1 boundary), then xc0, then xc1.  Phase E runs entirely on
            # the tail, where xc arrives exactly when needed and nothing
            # mid-stream ever waits on the descriptor-clogged hw queues.
            for b in range(2):
                for c in range(NCHUNK):
                    make_xg(b, c)
            for h in range(4):
                nc.sync.dma_start(
                    xc[:, h * NSH // 2 : (h + 1) * NSH // 2],
                    xc_ext[:, h * NSH // 2 : (h + 1) * NSH // 2],
                )

            # ---- constants ----
            bo = work.tile([P, 1], F32, tag="bo")
            nc.vector.tensor_copy(bo[:], wpack[:, 1792:1793])

            # ---- Gram accumulation (fp8 DoubleRow) ----
            g_ps = [None, None]
            gbf = [None, None]

            def gram_chunks(b, c_lo, c_hi):
                if g_ps[b] is None:
                    g_ps[b] = psg.tile([P, P], F32, tag="g", name=f"g_ps{b}")
                n_mm = CHUNK_SUB // 2
                for c in range(c_lo, c_hi):
                    xr = xg_tiles[b][c][:].rearrange("p (m q) -> p m q", q=2 * P)
                    for j in range(n_mm):
                        # software-interleaved pair block: per partition the
                        # 256 bytes are [A_c127, B_c127, ..., A_c0, B_c0]
                        # (A/B = the two k-subtiles, columns reversed per the
                        # HW SwInterleave contract).  The weights AP streams
                        # the storage order; the ifmap AP picks plane i at
                        # stride 2.  G comes out with reversed columns,
                        # absorbed by reversing wk's rows host-side.
                        blk = xr[:, j, :]
                        lhsT = blk.rearrange("p (qq two) -> p qq two", two=2)
                        rhs = blk.rearrange("p (qq two) -> p two qq", two=2)
                        nc.tensor.matmul(
                            g_ps[b][:], lhsT, rhs,
                            start=(c == 0 and j == 0),
                            stop=(c == NCHUNK - 1 and j == n_mm - 1),
                            perf_mode=DRSW,
                        )

            # ---- phase D: scores (PE), softmax (DVE/ACT), W_eff (PE) ----
            s_tiles = {}

            def d_scores(b):
                """gbf cast; a = G Wq; S_h = a_h^T Wk_h (quadrant-packed)."""
                gbf[b] = work.tile([P, P], BF, tag=f"gbf{b}", name=f"gbf{b}")
                nc.vector.tensor_copy(gbf[b][:], g_ps[b][:])
                a_ps = psd.tile([P, 512], F32, tag="d", name=f"a_ps{b}")
                a_sb = work.tile([P, 512], BF, tag=f"asb{b}", name=f"a_sb{b}")
                s_ps = psd.tile([P, 256], F32, tag="d", name=f"s_ps{b}")
                nc.tensor.matmul(a_ps[:], gbf[b][:], wq, start=True, stop=True)
                nc.vector.tensor_copy(a_sb[:], a_ps[:])
                for h in range(HEADS):
                    pb = 64 * (h % 2)
                    cg = 64 * (h // 2)
                    nc.tensor.matmul(
                        s_ps[pb : pb + 64, cg : cg + 64],
                        a_sb[:, h * 64 : (h + 1) * 64],
                        wk[:, h * 64 : (h + 1) * 64],
                        start=True, stop=True,
                    )
                s_tiles[b] = s_ps

            def d_softmax(b):
                """Per-group: exp(s - max) with fused row-sum, then scale.

                ACT Exp takes bias = -max (per-partition AP) and emits the
                row sum via accum_out in the same instruction, so the chain
                is DVE(max) -> ACT(exp+sum) -> DVE(recip) -> DVE(scale),
                pipelined across the 4 head-groups.
                """
                s_ps = s_tiles[b]
                negmax = work.tile([P, 4], F32, tag=f"nm{b}", name=f"negmax{b}")
                exp_sb = work.tile([P, 256], F32, tag=f"exp{b}", name=f"exp_sb{b}")
                sums = work.tile([P, 4], F32, tag=f"sums{b}", name=f"sums{b}")
                recip = work.tile([P, 4], F32, tag=f"recip{b}", name=f"recip{b}")
                attn = work.tile([P, 256], BF, tag=f"attn{b}", name=f"attn{b}")
                nc.vector.reduce_max(
                    negmax[:],
                    s_ps[:].rearrange("p (g j) -> p g j", j=64),
                    axis=mybir.AxisListType.X,
                    negate=True,
                )
                for g in range(4):
                    cg = 64 * g
                    nc.scalar.activation(
                        exp_sb[:, cg : cg + 64],
                        s_ps[:, cg : cg + 64],
                        EXP,
                        bias=negmax[:, g : g + 1],
                        scale=1.0,
                        accum_out=sums[:, g : g + 1],
                    )
                    nc.vector.reciprocal(recip[:, g : g + 1], sums[:, g : g + 1])
                    nc.vector.tensor_scalar_mul(
                        attn[:, cg : cg + 64],
                        exp_sb[:, cg : cg + 64],
                        recip[:, g : g + 1],
                    )
                return attn

            def d_weff(b, attn):
                """MT_h = attn_h^T WoT_h; W_eff = wv MT.

                Monolithic: 8 quadrant matmuls, ONE [128,256] cast, 4
                accumulating matmuls, one weff cast -- two cross-engine
                hops total.  (A per-group cast<->matmul ping-pong costs 8
                hops and serializes ~4us on the tail.)
                """
                mt_ps = psd.tile([P, 256], F32, tag="d", name=f"mt_ps{b}")
                mt_sb = work.tile([P, 256], BF, tag=f"mt{b}", name=f"mt_sb{b}")
                w_ps = psd.tile([P, 64], F32, tag="d", name=f"w_ps{b}")
                weff = work.tile([P, 64], BF, tag=f"weff{b}", name=f"weff{b}")
                for h in range(HEADS):
                    pb = 64 * (h % 2)
                    cg = 64 * (h // 2)
                    nc.tensor.matmul(
                        mt_ps[pb : pb + 64, cg : cg + 64],
                        attn[pb : pb + 64, cg : cg + 64],
                        wo[pb : pb + 64, cg : cg + 64],
                        start=True, stop=True,
                    )
                nc.vector.tensor_copy(mt_sb[:], mt_ps[:])
                for g in range(4):
                    nc.tensor.matmul(
                        w_ps[:],
                        wv[:, g * P : (g + 1) * P],
                        mt_sb[:, g * 64 : (g + 1) * 64],
                        start=(g == 0), stop=(g == 3),
                    )
                nc.vector.tensor_copy(weff[:], w_ps[:])
                return weff

            def phase_e(b, weff, t_lo, t_hi):
                """y_b = W_eff_b @ x_b + b_out, two 512-col chunks per PSUM
                tile via quadrant packing (out partitions 0-63 / 64-127)."""
                for t in range(t_lo, t_hi):
                    y_ps = psy.tile([P, 512], F32, tag="y", name=f"y_ps{b}_{t}")
                    for half in (0, 1):
                        j = 2 * t + half
                        nc.tensor.matmul(
                            y_ps[64 * half : 64 * half + 64, :],
                            weff[:],
                            xc[:, b * NSH + j * 512 : b * NSH + (j + 1) * 512],
                            start=True, stop=True,
                        )
                    y_sb = ypool.tile([P, 512], BF, tag="ysb", name=f"y_sb{b}_{t}")
                    # alternate ACT/DVE so consecutive pairs' bias-adds run
                    # in parallel instead of serializing on one engine
                    if t % 2 == 0:
                        nc.scalar.activation(
                            y_sb[:], y_ps[:],
                            mybir.ActivationFunctionType.Identity,
                            bias=bo[:, 0:1], scale=1.0,
                        )
                    else:
                        nc.vector.tensor_scalar_add(y_sb[:], y_ps[:], bo[:, 0:1])
                    dst = out_ext[:, (b * 4 + t) * 512 : (b * 4 + t + 1) * 512]
                    if b == 0:
                        nc.gpsimd.dma_start(dst, y_sb[:])
                    else:
                        qs[t % 2].dma_start(dst, y_sb[:])

            # ---- PE program order ----
            # gram0 -> gram1 back to back (stream-paced, no boundary gap:
            # D0's scores slot in after gram1's first chunk so the gbf0 cast
            # latency hides under data-paced matmuls).  The whole back half
            # (weff0/E0/weff1/E1) runs on the tail: weff0+E0 cover the
            # softmax1 ACT/DVE chain, and xc0/xc1 arrive (in that order)
            # right as phase E consumes them.
            gram_chunks(0, 0, NCHUNK)
            gram_chunks(1, 0, 1)
            d_scores(0)
            attn0 = d_softmax(0)
            gram_chunks(1, 1, NCHUNK - 1)
            weff0 = d_weff(0, attn0)   # fills the last chunk's data wait
            gram_chunks(1, NCHUNK - 1, NCHUNK)
            d_scores(1)
            attn1 = d_softmax(1)
            phase_e(0, weff0, 0, 4)    # covers the softmax1 ACT/DVE chain
            weff1 = d_weff(1, attn1)
            phase_e(1, weff1, 0, 4)

    nc.compile()
    return nc


def _get_nc():
    global _CACHED_NC
    if _CACHED_NC is None:
        _CACHED_NC = build_nc()
    return _CACHED_NC


def make_in_maps(x, w_qkv, w_out, b_out):
    x = np.ascontiguousarray(x, dtype=np.float32)
    w_qkv = np.asarray(w_qkv, dtype=np.float32)
    w_out = np.asarray(w_out, dtype=np.float32)
    b_out = np.asarray(b_out, dtype=np.float32)
    xf = x.reshape(2, P, N_TOT)

    # full x, fp8, DoubleRowSwInterleave layout: subtile pairs (2t, 2t+1)
    # interleaved per column with columns reversed:
    # [p, (b, t, qq, which)] where element = x^T[subtile 2t+which][p, 127-qq]
    arr = (
        xf.transpose(0, 2, 1)            # (2, n, c)
        .reshape(2, SUB, P, P)           # (2, m, p, c)
    )
    inter = np.stack(
        [arr[:, 0::2, :, ::-1], arr[:, 1::2, :, ::-1]], axis=-1
    )                                    # (2, t, p, qq, which)
    xg_h = np.ascontiguousarray(
        inter.transpose(2, 0, 1, 3, 4).reshape(P, 2 * SUB * P)
    ).astype(f8)

    wpack = np.zeros((P, WCOLS), np.float32)
    wpack[:, 0:512] = w_qkv[:512].T * SCALE
    # rows reversed: the SwInterleave Gram produces G with reversed columns,
    # so a = G' Wq has reversed rows; reversing wk's contraction rows undoes
    # it exactly (G is symmetric).
    wpack[:, 512:1024] = w_qkv[512:1024].T[::-1, :]
    wpack[:, 1024:1536] = (
        (w_qkv[1024:] / N_TOT).reshape(4, P, P).transpose(1, 0, 2).reshape(P, 512)
    )
    for h in range(HEADS):
        wpack[
            64 * (h % 2) : 64 * (h % 2) + 64,
            1536 + 64 * (h // 2) : 1536 + 64 * (h // 2) + 64,
        ] = w_out[:, h * 64 : (h + 1) * 64].T
    wpack[:, 1792] = np.concatenate([b_out, b_out])
    wpack_h = wpack.astype(bf16)

    in_maps = []
    for c in range(NCORES):
        # own output shard, bf16, [c, (b, n)]
        xc_h = np.ascontiguousarray(
            xf[:, :, c * NSH : (c + 1) * NSH].transpose(1, 0, 2).reshape(P, 2 * NSH)
        ).astype(bf16)
        in_maps.append({"xg": xg_h, "xc": xc_h, "wpack": wpack_h})
    return in_maps


def assemble_output(results):
    # out layout: [p = 64*half + row, (b, pair t, 512)]; spatial column of
    # (b, t, half, col) is shard_base + (2t + half)*512 + col.
    y = np.empty((2, 64, N_TOT), np.float32)
    for c in range(NCORES):
        o = np.asarray(results[c]["out"]).astype(np.float32)  # [128, 4096]
        for b in range(2):
            for t in range(4):
                blk = o[:, (b * 4 + t) * 512 : (b * 4 + t + 1) * 512]
                y[b, :, c * NSH + 2 * t * 512 : c * NSH + (2 * t + 1) * 512] = blk[:64]
                y[b, :, c * NSH + (2 * t + 1) * 512 : c * NSH + (2 * t + 2) * 512] = (
                    blk[64:]
                )
    return y.reshape(2, 64, 32, 32, 32)


def kernel(**inputs):
    in_maps = make_in_maps(
        inputs["x"], inputs["w_qkv"], inputs["w_out"], inputs["b_out"]
    )
    nc = _get_nc()
    res = run_bass_kernel_spmd(nc, in_maps, core_ids=list(range(NCORES)))
    return assemble_output(res.results)
